# revision 1
# baseline (speedup 1.0000x reference)
"""CapsuleNet forward kernel for 8 Trainium2 NeuronCores.

Data-parallel over batch (64 images / core); the routing b_ij batch-mean
uses an AllReduce per iteration.  u_hat is never materialized: s_j and the
agreement mean are computed directly against W from the 9216-dim flattened
capsule vector u.

Per-core pipeline:
  conv1  : one K=81 matmul per output tile (im2col built by a single
           strided DMA from DRAM, 2240B segments, 8 garbage cols/row that
           are cropped during the ReLU copy; conv1 bias folded into the
           ReLU's bias operand)
  conv2  : 324 accumulating K=128 matmuls (81 taps x 2 ci chunks) per co
           chunk over the full local batch (5 image-aligned PSUM banks)
  capsule: scatter-transpose conv2 output to u2T[b, f] (f = co*36+s),
           squash over 8-elem groups, PE-transpose to u2R[f, b]
  routing: s_j^T = (c-scaled W)^T @ u2, 72 K-tile accumulation;
           agreement mean m = sum_{o,i} W .* (v2^T @ u2) via rank-64
           matmul + DVE mult/group-reduce + selector matmuls;
           AllReduce(m) -> b_ij update -> softmax.
"""

import numpy as np
import ml_dtypes

import concourse.bacc as bacc
import concourse.bass as bass
import concourse.mybir as mybir
import concourse.tile as tile
from concourse.bass_utils import run_bass_kernel_spmd

F32 = mybir.dt.float32
BF16 = mybir.dt.bfloat16
MUL = mybir.AluOpType.mult
ADD = mybir.AluOpType.add
MAX = mybir.AluOpType.max
AXX = mybir.AxisListType.X
ACT = mybir.ActivationFunctionType

NCORES = 8
B = 512
BL = B // NCORES        # 64 images per core
SB = 16                 # conv1 im2col sub-batch
NSB = BL // SB
J = 560                 # 20 rows x 28 cols (8 garbage cols/row)
JC = 400                # compact 20x20 conv1 output per image
R, C, O, I = 1152, 10, 16, 8
F = R * I               # 9216
CO = C * O              # 160
KT = F // 128           # 72
S2 = 36                 # 6x6 conv2 positions per image
N2 = BL * S2
BCH = [(0, 14), (14, 14), (28, 14), (42, 14), (56, 8)]
NIT = 3


def _sub(ap, off, dims):
    """Arbitrary strided view (offset in elements, dims=[[step,count],..])."""
    return bass.AP(ap.tensor, ap.offset + off, [list(d) for d in dims])


def _pp(ap):
    """Partition pitch (elements per partition row) of an SBUF AP."""
    return ap.ap[0][0]


def build_nc(for_sim=False, reps=1):
    nc = bacc.Bacc("TRN2", target_bir_lowering=False, debug=False,
                   num_devices=1 if for_sim else NCORES)
    nc._for_sim = for_sim

    xin = nc.dram_tensor("xin", [BL * 784 + 8], BF16, kind="ExternalInput").ap()
    w1t = nc.dram_tensor("w1t", [81, 256], BF16, kind="ExternalInput").ap()
    b1 = nc.dram_tensor("b1", [128, 2], F32, kind="ExternalInput").ap()
    w2s = nc.dram_tensor("w2s", [162, 128, 256], BF16, kind="ExternalInput").ap()
    b2 = nc.dram_tensor("b2", [128, 2], F32, kind="ExternalInput").ap()
    wlb = nc.dram_tensor("wlb", [F, CO], BF16, kind="ExternalInput").ap()
    wtf = nc.dram_tensor("wtf", [CO, F], F32, kind="ExternalInput").ap()
    sel8 = nc.dram_tensor("sel8", [128, 8], F32, kind="ExternalInput").ap()
    sel2 = nc.dram_tensor("sel2", [32, 2], F32, kind="ExternalInput").ap()
    eyeb = nc.dram_tensor("eyeb", [64, 64], BF16, kind="ExternalInput").ap()
    eyef = nc.dram_tensor("eyef", [16, 16], F32, kind="ExternalInput").ap()
    out = nc.dram_tensor("out", [BL, CO], F32, kind="ExternalOutput").ap()

    selr = nc.dram_tensor("selr", [8, 128, 128], BF16, kind="ExternalInput").ap()
    cc_in = nc.dram_tensor("cc_in", [C, R], F32)
    cc_out = nc.dram_tensor("cc_out", [C, R], F32,
                            addr_space="Local" if for_sim else "Shared")
    vd = nc.dram_tensor("vd", [2, 128, N2], F32)       # conv2 out bounce

    with tile.TileContext(nc, num_cores=NCORES) as tc:
        for _rep in range(reps):
            _body(tc, nc, xin, w1t, b1, w2s, b2, wlb, wtf, sel8, sel2,
                  eyeb, eyef, selr, out, cc_in, cc_out, vd)
    nc.compile()
    return nc


def _body(tc, nc, xin, w1t, b1, w2s, b2, wlb, wtf, sel8, sel2, eyeb, eyef,
          selr, out, cc_in, cc_out, vd):
    with tc.tile_pool(name="const", bufs=1) as pc, \
         tc.tile_pool(name="upers", bufs=1) as pU:

        w1t_sb = pc.tile([81, 256], BF16, tag="w1t")
        nc.sync.dma_start(w1t_sb[:], w1t)
        b1_sb = pc.tile([128, 2], F32, tag="b1")
        nc.sync.dma_start(b1_sb[:], b1)
        b2_sb = pc.tile([128, 2], F32, tag="b2")
        nc.sync.dma_start(b2_sb[:], b2)
        sel8_sb = pc.tile([128, 8], F32, tag="sel8")
        nc.sync.dma_start(sel8_sb[:], sel8)
        sel2_sb = pc.tile([32, 2], F32, tag="sel2")
        nc.sync.dma_start(sel2_sb[:], sel2)
        eyeb_sb = pc.tile([64, 64], BF16, tag="eyeb")
        nc.sync.dma_start(eyeb_sb[:], eyeb)
        eyef_sb = pc.tile([16, 16], F32, tag="eyef")
        nc.sync.dma_start(eyef_sb[:], eyef)
        selr_sb = pc.tile([128, 8 * 128], BF16, tag="selr")
        nc.sync.dma_start(
            _sub(selr_sb[:], 0, [[_pp(selr_sb[:]), 128], [128, 8], [1, 128]]),
            _sub(selr, 0, [[128, 128], [128 * 128, 8], [1, 128]]))

        u2Tb = pU.tile([BL, F], BF16, tag="u2Tb")       # squashed u, b-major
        u2R = pU.tile([128, KT * BL], BF16, tag="u2R")  # squashed u, f-major

        # ============ Phase A: conv1 + conv2 + capsule formation ===========
        with tc.tile_pool(name="uT", bufs=1) as pT:
            u2T = pT.tile([BL, F], F32, tag="u2T")      # raw capsules, b-major

            with tc.tile_pool(name="pA", bufs=1) as pA, \
                 tc.tile_pool(name="pH", bufs=1) as pH, \
                 tc.tile_pool(name="pW2", bufs=8) as pW2, \
                 tc.tile_pool(name="pV", bufs=1) as pV, \
                 tc.tile_pool(name="ps1", bufs=2, space="PSUM") as ps1, \
                 tc.tile_pool(name="ps2", bufs=1, space="PSUM") as ps2:

                h1 = [pH.tile([128, BL * JC], BF16, tag=f"h1_{kc}",
                              name=f"h1_{kc}") for kc in range(2)]

                flip = 0
                for sb in range(NSB):
                    A = pA.tile([81, SB * J], BF16, tag="A")
                    pa = _pp(A[:])
                    for kh in range(9):
                        src = _sub(xin, sb * SB * 784 + 28 * kh,
                                   [[1, 9], [784, SB], [1, J]])
                        dst = _sub(A[:], 9 * kh * pa,
                                   [[pa, 9], [J, SB], [1, J]])
                        nc.sync.dma_start(dst, src)

                    for mc in range(2):
                        lhsT = w1t_sb[:, mc * 128:(mc + 1) * 128]
                        for bi in range(SB):
                            for hf in range(2):
                                ps = ps1.tile([128, 280], F32, tag="c1ps")
                                rhs = A[:, bi * J + hf * 280: bi * J + hf * 280 + 280]
                                nc.tensor.matmul(ps[:], lhsT, rhs,
                                                 start=True, stop=True)
                                doff = (sb * SB + bi) * JC + hf * 200
                                dstc = _sub(h1[mc][:], doff,
                                            [[_pp(h1[mc][:]), 128], [20, 10], [1, 20]])
                                srcc = _sub(ps[:], 0,
                                            [[_pp(ps[:]), 128], [28, 10], [1, 20]])
                                bb = b1_sb[:, mc:mc + 1]
                                if flip % 2 == 0:
                                    nc.vector.tensor_scalar(dstc, srcc, bb, 0.0,
                                                            op0=ADD, op1=MAX)
                                else:
                                    nc.scalar.activation(dstc, srcc, ACT.Relu,
                                                         bias=bb)
                                flip += 1

                # conv2
                for mc in range(2):
                    pss = [ps2.tile([128, nb * S2], F32, tag=f"c2ps{i}",
                                    name=f"c2ps{i}_{mc}")
                           for i, (_, nb) in enumerate(BCH)]
                    for kc in range(2):
                        for khw in range(81):
                            kh2, kw2 = khw // 9, khw % 9
                            wch = pW2.tile([128, 256], BF16, tag="wch")
                            nc.sync.dma_start(wch[:], w2s[khw * 2 + kc])
                            lhsT = wch[:, mc * 128:(mc + 1) * 128]
                            for ic, (b0, nb) in enumerate(BCH):
                                rhs = _sub(h1[kc][:], b0 * JC + 20 * kh2 + kw2,
                                           [[_pp(h1[kc][:]), 128],
                                            [JC, nb], [40, 6], [2, 6]])
                                nc.tensor.matmul(
                                    pss[ic][:], lhsT, rhs,
                                    start=(kc == 0 and khw == 0),
                                    stop=(kc == 1 and khw == 80))
                    v = pV.tile([128, N2], F32, tag="v")
                    for ic, (b0, nb) in enumerate(BCH):
                        nc.vector.tensor_scalar(v[:, b0 * S2:(b0 + nb) * S2],
                                                pss[ic][:], b2_sb[:, mc:mc + 1],
                                                None, op0=ADD)
                    # bounce via DRAM: SBUF-side DMA APs need the partition
                    # dim outermost, so the (co,b)->(b,co) transpose is done
                    # on the DRAM side
                    nc.sync.dma_start(vd.ap()[mc], v[:])
                    usrc = _sub(vd.ap(), mc * 128 * N2,
                                [[S2, BL], [N2, 128], [1, S2]])
                    udst = _sub(u2T[:], mc * 128 * S2,
                                [[_pp(u2T[:]), BL], [S2, 128], [1, S2]])
                    nc.sync.dma_start(udst, usrc)

            # ============ squash u (capsule groups of 8) ===================
            with tc.tile_pool(name="squ", bufs=1) as pq:
                sqr = pq.tile([BL, F], F32, tag="sqr")
                nc.vector.tensor_mul(sqr[:], u2T[:], u2T[:])
                sq = pq.tile([BL, R], F32, tag="sq")
                nc.vector.tensor_reduce(sq[:],
                                        sqr[:].rearrange("p (r i) -> p r i", i=I),
                                        axis=AXX, op=ADD)
                srt = pq.tile([BL, R], F32, tag="srt")
                nc.scalar.sqrt(srt[:], sq[:])
                d1 = pq.tile([BL, R], F32, tag="d1")
                nc.vector.tensor_scalar(d1[:], sq[:], 1.0, None, op0=ADD)
                d2 = pq.tile([BL, R], F32, tag="d2")
                nc.vector.tensor_mul(d2[:], d1[:], srt[:])
                rc = pq.tile([BL, R], F32, tag="rc")
                nc.vector.reciprocal(rc[:], d2[:])
                g = pq.tile([BL, R], F32, tag="g")
                nc.vector.tensor_mul(g[:], sq[:], rc[:])
                # u2Tb = u2T * g, one strided pass per capsule element
                ppu = _pp(u2T[:])
                ppb = _pp(u2Tb[:])
                for i in range(I):
                    nc.vector.tensor_tensor(
                        _sub(u2Tb[:], i, [[ppb, BL], [I, R]]),
                        _sub(u2T[:], i, [[ppu, BL], [I, R]]),
                        g[:], op=MUL)

        # ============ u2R = transpose(u2Tb) ================================
        with tc.tile_pool(name="ptr", bufs=2, space="PSUM") as ptr:
            for t in range(KT):
                pst = ptr.tile([128, BL], BF16, tag="tr")
                nc.tensor.transpose(pst[:], u2Tb[:, t * 128:(t + 1) * 128],
                                    eyeb_sb[:])
                nc.vector.tensor_copy(u2R[:, t * BL:(t + 1) * BL], pst[:])

        # ============ routing ==============================================
        with tc.tile_pool(name="pB", bufs=1) as pB, \
             tc.tile_pool(name="pBs", bufs=2) as pBs, \
             tc.tile_pool(name="psq2", bufs=1) as pq, \
             tc.tile_pool(name="psB", bufs=2, space="PSUM") as psB, \
             tc.tile_pool(name="psS", bufs=1, space="PSUM") as psS:

            wsb = pB.tile([128, KT * CO], BF16, tag="wsb")
            wsrc = _sub(wlb, 0, [[CO, 128], [128 * CO, KT], [1, CO]])
            wdst = _sub(wsb[:], 0, [[_pp(wsb[:]), 128], [CO, KT], [1, CO]])
            nc.sync.dma_start(wdst, wsrc)
            wt0 = pB.tile([128, F], F32, tag="wt0")
            nc.sync.dma_start(wt0[:], wtf[0:128])
            wt1 = pB.tile([32, F], F32, tag="wt1")
            nc.sync.dma_start(wt1[:], wtf[128:160])
            wp = pB.tile([128, KT * CO], BF16, tag="wp")
            cE = pB.tile([128, KT * C], BF16, tag="cE")
            cTr = pB.tile([128, 9 * C], BF16, tag="cTr")
            mAll = pB.tile([8, R], F32, tag="mAll")
            mAll2 = pB.tile([2, R], F32, tag="mAll2")
            bijA = pB.tile([C, R], F32, tag="bijA")
            bijB = pB.tile([C, R], F32, tag="bijB")
            csm = pB.tile([C, R], F32, tag="csm")
            v2T = pB.tile([BL, CO], F32, tag="v2T")
            v2Tb = pB.tile([BL, CO], BF16, tag="v2Tb")
            msum = pB.tile([C, R], F32, tag="msum")

            lam = 1.0 / R
            for it in range(NIT):
                if it > 0:
                    # cTr[r%128, q*10+c] = csm[c, r]  (PE transpose, 9 blocks)
                    for q in range(9):
                        pst = psB.tile([128, C], F32, tag="ctr", name="ctr", bufs=1)
                        nc.tensor.transpose(pst[:],
                                            csm[:, q * 128:(q + 1) * 128],
                                            eyef_sb[0:C, 0:C])
                        nc.vector.tensor_copy(cTr[:, q * C:(q + 1) * C], pst[:])
                    # cE[8r''+i, (8t2+t1)*10+c] = cTr[16*t1+r'', t2*10+c]
                    # via selector matmuls: SEL_t1[k, p] = (k == 16*t1 + p//8)
                    for t1 in range(8):
                        pse = psB.tile([128, 9 * C], F32, tag="cexp", name="cexp", bufs=1)
                        nc.tensor.matmul(pse[:],
                                         selr_sb[:, t1 * 128:(t1 + 1) * 128],
                                         cTr[:], start=True, stop=True)
                        nc.vector.tensor_copy(
                            _sub(cE[:], t1 * C,
                                 [[_pp(cE[:]), 128], [8 * C, 9], [1, C]]),
                            pse[:])
                    # wp = wsb * cE, one strided pass per o
                    ppw = _pp(wp[:])
                    pps = _pp(wsb[:])
                    for o in range(O):
                        nc.vector.tensor_tensor(
                            _sub(wp[:], o, [[ppw, 128], [CO, KT], [O, C]]),
                            _sub(wsb[:], o, [[pps, 128], [CO, KT], [O, C]]),
                            cE[:].rearrange("p (t c) -> p t c", c=C), op=MUL)

                # s_j^T [b, co] over 72 accumulating K-tiles
                wcur = wsb if it == 0 else wp
                ssum = psS.tile([BL, CO], F32, tag="ssum")
                for t in range(KT):
                    nc.tensor.matmul(ssum[:], u2R[:, t * BL:(t + 1) * BL],
                                     wcur[:, t * CO:(t + 1) * CO],
                                     start=(t == 0), stop=(t == KT - 1))

                # v2 = squash(s) over o-groups of 16 (iter0 folds the 1/R scale)
                ssb = pq.tile([BL, CO], F32, tag="ssb")
                nc.vector.tensor_copy(ssb[:], ssum[:])
                svr = pq.tile([BL, CO], F32, tag="svr")
                nc.vector.tensor_mul(svr[:], ssb[:], ssb[:])
                sqv = pq.tile([BL, C], F32, tag="sqv")
                nc.vector.tensor_reduce(sqv[:],
                                        svr[:].rearrange("p (c o) -> p c o", o=O),
                                        axis=AXX, op=ADD)
                if it == 0:
                    nc.vector.tensor_scalar(sqv[:], sqv[:], lam * lam, None, op0=MUL)
                srtv = pq.tile([BL, C], F32, tag="srtv")
                nc.scalar.sqrt(srtv[:], sqv[:])
                dv1 = pq.tile([BL, C], F32, tag="dv1")
                nc.vector.tensor_scalar(dv1[:], sqv[:], 1.0, None, op0=ADD)
                dv2 = pq.tile([BL, C], F32, tag="dv2")
                nc.vector.tensor_mul(dv2[:], dv1[:], srtv[:])
                rcv = pq.tile([BL, C], F32, tag="rcv")
                nc.vector.reciprocal(rcv[:], dv2[:])
                gv = pq.tile([BL, C], F32, tag="gv")
                nc.vector.tensor_mul(gv[:], sqv[:], rcv[:])
                if it == 0:
                    nc.vector.tensor_scalar(gv[:], gv[:], lam, None, op0=MUL)
                ppv = _pp(v2T[:])
                pps2 = _pp(ssb[:])
                for o in range(O):
                    nc.vector.tensor_tensor(
                        _sub(v2T[:], o, [[ppv, BL], [O, C]]),
                        _sub(ssb[:], o, [[pps2, BL], [O, C]]),
                        gv[:], op=MUL)

                if it == NIT - 1:
                    nc.sync.dma_start(out, v2T[:])
                    break

                nc.vector.tensor_copy(v2Tb[:], v2T[:])
                # m[c, r] = sum_{o,i} Wt[(c,o),(r,i)] * (v2^T @ u2)[(c,o),(r,i)]
                for mc2 in range(2):
                    npart = 128 if mc2 == 0 else 32
                    ncls = 8 if mc2 == 0 else 2
                    lhs = v2Tb[:, mc2 * 128: mc2 * 128 + npart]
                    selt = (sel8_sb if mc2 == 0 else sel2_sb)[0:npart, 0:ncls]
                    wtt = wt0 if mc2 == 0 else wt1
                    for nch in range(18):
                        f0 = nch * 512
                        tps = psB.tile([128, 512], F32, tag="tprime")
                        nc.tensor.matmul(tps[0:npart, :], lhs,
                                         u2Tb[:, f0:f0 + 512],
                                         start=True, stop=True)
                        pm = pBs.tile([128, 512], F32, tag="pm")
                        nc.vector.tensor_tensor(pm[0:npart, :],
                                                wtt[0:npart, f0:f0 + 512],
                                                tps[0:npart, :], op=MUL)
                        pr = pBs.tile([128, 64], F32, tag="pr")
                        nc.vector.tensor_reduce(
                            pr[0:npart, :],
                            pm[0:npart, :].rearrange("p (r i) -> p r i", i=I),
                            axis=AXX, op=ADD)
                        mo = psB.tile([16, 64], F32, tag="mo", bufs=2)
                        nc.tensor.matmul(mo[0:ncls, :], selt, pr[0:npart, :],
                                         start=True, stop=True)
                        mtgt = mAll if mc2 == 0 else mAll2
                        nc.vector.tensor_copy(
                            mtgt[0:ncls, f0 // I: f0 // I + 64],
                            mo[0:ncls, :])

                nc.sync.dma_start(cc_in.ap()[0:8], mAll[:])
                nc.sync.dma_start(cc_in.ap()[8:10], mAll2[:])
                if getattr(nc, "_for_sim", False):
                    nc.sync.dma_start(cc_out.ap(), cc_in.ap())
                else:
                    nc.gpsimd.collective_compute(
                        "AllReduce", ADD,
                        replica_groups=[list(range(NCORES))],
                        ins=[cc_in.ap()], outs=[cc_out.ap()])
                nc.sync.dma_start(msum[:], cc_out.ap())
                bij = bijA if it == 0 else bijB
                if it == 0:
                    nc.vector.tensor_scalar(bij[:], msum[:], 1.0 / B, None, op0=MUL)
                else:
                    nc.vector.tensor_scalar(bij[:], msum[:], 1.0 / B, None, op0=MUL)
                    nc.vector.tensor_add(bij[:], bij[:], bijA[:])
                # softmax over routes (free dim)
                rmax = pq.tile([C, 1], F32, tag="rmax")
                nc.vector.tensor_reduce(rmax[:], bij[:], axis=AXX, op=MAX)
                nrm = pq.tile([C, 1], F32, tag="nrm")
                nc.vector.tensor_scalar(nrm[:], rmax[:], -1.0, None, op0=MUL)
                nc.scalar.activation(csm[:], bij[:], ACT.Exp, bias=nrm[:])
                rsm = pq.tile([C, 1], F32, tag="rsm")
                nc.vector.tensor_reduce(rsm[:], csm[:], axis=AXX, op=ADD)
                rrc = pq.tile([C, 1], F32, tag="rrc")
                nc.vector.reciprocal(rrc[:], rsm[:])
                nc.vector.tensor_scalar(csm[:], csm[:], rrc[:], None, op0=MUL)


# ------------------------- host side ---------------------------------------
_CACHE = {}


def kernel(x, conv1_w, conv1_b, conv2_w, conv2_b, W):
    if "nc" not in _CACHE:
        _CACHE["nc"] = build_nc()
    nc = _CACHE["nc"]

    bf = ml_dtypes.bfloat16
    xf = np.ascontiguousarray(np.asarray(x, np.float32).reshape(B, 784))
    w1 = np.ascontiguousarray(
        np.asarray(conv1_w, np.float32).reshape(256, 81).T).astype(bf)
    b1v = np.asarray(conv1_b, np.float32).reshape(2, 128).T.copy()
    w2 = np.asarray(conv2_w, np.float32).reshape(256, 256, 81)
    w2 = np.ascontiguousarray(w2.transpose(2, 1, 0)).reshape(162, 128, 256).astype(bf)
    b2v = np.asarray(conv2_b, np.float32).reshape(2, 128).T.copy()
    Wf = np.asarray(W, np.float32)
    wl = np.ascontiguousarray(Wf.transpose(0, 3, 1, 2)).reshape(F, CO).astype(bf)
    wt = np.ascontiguousarray(Wf.transpose(1, 2, 0, 3)).reshape(CO, F).astype(np.float32)
    s8 = np.zeros((128, 8), np.float32)
    s8[np.arange(128), np.arange(128) // 16] = 1.0
    s2m = np.zeros((32, 2), np.float32)
    s2m[np.arange(32), np.arange(32) // 16] = 1.0
    srn = np.zeros((8, 128, 128), np.float32)
    for t1 in range(8):
        srn[t1, 16 * t1 + np.arange(128) // 8, np.arange(128)] = 1.0

    shared = {
        "w1t": w1, "b1": b1v, "w2s": w2, "b2": b2v, "wlb": wl, "wtf": wt,
        "sel8": s8, "sel2": s2m, "selr": srn.astype(bf),
        "eyeb": np.eye(64).astype(bf), "eyef": np.eye(16, dtype=np.float32),
    }
    in_maps = []
    for c in range(NCORES):
        xs = np.zeros(BL * 784 + 8, bf)
        xs[:BL * 784] = xf[c * BL:(c + 1) * BL].reshape(-1).astype(bf)
        in_maps.append({"xin": xs, **shared})
    res = run_bass_kernel_spmd(nc, in_maps, list(range(NCORES)), trace=False)
    outs = [res.results[c]["out"] for c in range(NCORES)]
    return np.concatenate(outs, axis=0).reshape(B, C, O).astype(np.float32)



# revision 21
# speedup vs baseline: 1.3256x; 1.3256x over previous
"""CapsuleNet forward kernel for 8 Trainium2 NeuronCores.

Data-parallel over batch (64 images / core); the routing b_ij batch-mean
uses an AllReduce per iteration.  u_hat is never materialized.

Per-core pipeline (v3):
  conv1  : x replicated into 81 partitions by a strided DMA (partition
           p = 9*kh+kw holds x shifted by 28*kh+kw), one K=81 matmul per
           (image, co-chunk) streaming only the 400 valid positions;
           ReLU+bias copies split across DVE/Act.
  conv2  : 324 accumulating K=128 matmuls per co chunk; weights streamed
           as 18-tap [128, 2304] chunks holding only the used co half;
           output bias-added to bf16 and bounced through DRAM with a
           transposed scatter-write (idle Act queue) to form
           u2T[b, co*36+s] in bf16.  The squash of each co-half runs on
           DVE/Act/Pool underneath the other half's matmuls.
  squash : u2T scaled by g = sqrt(|u|^2)/(1+|u|^2) in place (bf16);
           PE-transpose to u2R[f, b].
  routing: all CO-vectors use (o, c) column order.
           s_j^T = (c-scaled W)^T @ u2 via 72 K-tile accumulation.
           Agreement in f-major: tpsT[f,(o,c)] = u2T_chunk^T @ v2Tb via
           72 rank-64 matmuls; pm = wsb .* tpsT; both group reductions
           (i over partition groups of 8, o over column slices) folded
           into 32 accumulating selector matmuls; AllReduce in the
           [16, 720] layout; b_ij replicated to f-major by a selector
           matmul; exp on Act with the softmax normalizer folded into
           the v-squash scale (no max subtraction needed: |b| is tiny).
"""

import numpy as np
import ml_dtypes

import concourse.bacc as bacc
import concourse.bass as bass
import concourse.bass_isa as bass_isa
import concourse.mybir as mybir
import concourse.tile as tile
from concourse.bass_utils import run_bass_kernel_spmd

F32 = mybir.dt.float32
BF16 = mybir.dt.bfloat16
MUL = mybir.AluOpType.mult
ADD = mybir.AluOpType.add
MAX = mybir.AluOpType.max
AXX = mybir.AxisListType.X
ACT = mybir.ActivationFunctionType

NCORES = 8
B = 512
BL = B // NCORES        # 64 images per core
R, C, O, I = 1152, 10, 16, 8
F = R * I               # 9216
CO = C * O              # 160
KT = F // 128           # 72
JC = 400                # compact 20x20 conv1 output per image
S2 = 36                 # 6x6 conv2 positions per image
N2 = BL * S2
XC = 4                  # conv1 im2col images per chunk
NXC = BL // XC
QW = 18                 # conv2 weight (kc,khw)-pairs per DMA chunk
NWCH = 162 // QW        # 9
BCH = [(0, 14), (14, 14), (28, 14), (42, 14), (56, 8)]
NIT = 3
FH = F // 2             # 4608
RH = R // 2             # 576


def _sub(ap, off, dims):
    """Arbitrary strided view (offset in elements, dims=[[step,count],..])."""
    return bass.AP(ap.tensor, ap.offset + off, [list(d) for d in dims])


def _pp(ap):
    """Partition pitch (elements per partition row) of an SBUF AP."""
    return ap.ap[0][0]


def build_nc(for_sim=False, reps=1, nit=NIT, use_collective=True,
             stride_probe=False):
    nc = bacc.Bacc("TRN2", target_bir_lowering=False, debug=False,
                   num_devices=1 if for_sim else NCORES)
    nc._for_sim = for_sim or not use_collective
    nc._nit = nit
    nc._stride_probe = stride_probe

    xin = nc.dram_tensor("xin", [BL * 784 + 256], BF16, kind="ExternalInput").ap()
    w1t = nc.dram_tensor("w1t", [81, 256], BF16, kind="ExternalInput").ap()
    b1 = nc.dram_tensor("b1", [128, 2], F32, kind="ExternalInput").ap()
    w2m = nc.dram_tensor("w2m", [2 * NWCH, 128, QW * 128], BF16,
                         kind="ExternalInput").ap()
    b2 = nc.dram_tensor("b2", [128, 2], F32, kind="ExternalInput").ap()
    wlbr = nc.dram_tensor("wlbr", [128, KT * CO], BF16, kind="ExternalInput").ap()
    selA = nc.dram_tensor("selA", [128, 16], BF16, kind="ExternalInput").ap()
    selR = nc.dram_tensor("selR", [16, 128], F32, kind="ExternalInput").ap()
    eyeb = nc.dram_tensor("eyeb", [64, 64], BF16, kind="ExternalInput").ap()
    out = nc.dram_tensor("out", [BL, CO], F32, kind="ExternalOutput").ap()

    cc_in = nc.dram_tensor("cc_in", [16, KT * C], F32)
    cc_out = nc.dram_tensor("cc_out", [16, KT * C], F32,
                            addr_space="Local" if for_sim else "Shared")
    vd2 = nc.dram_tensor("vd2", [BL, F], BF16)   # conv2 out bounce, u2T layout

    with tile.TileContext(nc, num_cores=NCORES) as tc:
        for _rep in range(reps):
            _body(tc, nc, xin, w1t, b1, w2m, b2, wlbr, selA, selR,
                  eyeb, out, cc_in, cc_out, vd2)
    nc.compile()
    return nc


def _body(tc, nc, xin, w1t, b1, w2m, b2, wlbr, selA, selR, eyeb,
          out, cc_in, cc_out, vd2):
    DVE, SCA, POOL, SYNC = nc.vector, nc.scalar, nc.gpsimd, nc.sync

    with tc.tile_pool(name="const", bufs=1) as pc, \
         tc.tile_pool(name="upers", bufs=1) as pU:

        w1t_sb = pc.tile([81, 256], BF16, tag="w1t")
        nc.sync.dma_start(w1t_sb[:], w1t)
        b1_sb = pc.tile([128, 2], F32, tag="b1")
        nc.sync.dma_start(b1_sb[:], b1)
        b2_sb = pc.tile([128, 2], F32, tag="b2")
        selA_sb = pc.tile([128, 16], BF16, tag="selA")
        selR_sb = pc.tile([16, 128], F32, tag="selR")
        eyeb_sb = pc.tile([64, 64], BF16, tag="eyeb")
        on128 = pc.tile([128, 1], BF16, tag="on128")
        DVE.memset(on128[:], 0.125)    # folds the x8 i-replication
        on64 = pc.tile([1, BL], F32, tag="on64")
        DVE.memset(on64[:], 1.0)

        wsb = pU.tile([128, KT * CO], BF16, tag="wsb")  # W [f, (o,c)]
        u2T = pU.tile([BL, F], BF16, tag="u2T")         # capsules, b-major
        u2R = pU.tile([128, KT * BL], BF16, tag="u2R")  # squashed u, f-major

        def relu_copy(eng, dst, src, bb):
            if eng is SCA:
                SCA.activation(dst, src, ACT.Relu, bias=bb)
            else:
                eng.tensor_scalar(dst, src, bb, 0.0, op0=ADD, op1=MAX)

        # conv1 ReLU-copy engine pattern: Act 55% / DVE 45%
        PAT = [SCA, DVE, SCA, DVE, SCA, DVE, SCA, DVE, SCA, DVE,
               SCA, SCA, DVE, SCA, DVE, SCA, DVE, SCA, DVE, SCA]

        with tc.tile_pool(name="squ", bufs=1) as pq:
            sqr = pq.tile([BL, FH], BF16, tag="sqr")    # per-half u*u
            sq = pq.tile([BL, R], F32, tag="sq")
            srt = pq.tile([BL, R], F32, tag="srt")
            d1 = pq.tile([BL, R], F32, tag="d1")

            def squash_half(h):
                """Squash capsules [h*RH, (h+1)*RH) of u2T, in place.
                Runs on DVE/Act/Pool only (PE-free)."""
                f0, r0 = h * FH, h * RH
                DVE.tensor_tensor(sqr[:, 0:2304], u2T[:, f0:f0 + 2304],
                                  u2T[:, f0:f0 + 2304], op=MUL)
                SCA.activation(sqr[:, 2304:FH], u2T[:, f0 + 2304:f0 + FH],
                               ACT.Square)
                sqh = sq[:, r0:r0 + RH]
                nc.vector.tensor_reduce(
                    sqh, sqr[:].rearrange("p (r i) -> p r i", i=I),
                    axis=AXX, op=ADD)
                srth = srt[:, r0:r0 + RH]
                SCA.sqrt(srth, sqh)
                d1h = d1[:, r0:r0 + RH]
                POOL.tensor_scalar(d1h, sqh, 1.0, None, op0=ADD)
                DVE.reciprocal(d1h, d1h)
                DVE.tensor_mul(srth, srth, d1h)   # g = sqrt(sq)/(1+sq)
                ppu = _pp(u2T[:])
                ppg = _pp(srt[:])
                for i in range(I):
                    eng = DVE if i % 4 < 2 else POOL
                    eng.tensor_tensor(
                        _sub(u2T[:], f0 + i, [[ppu, BL], [I, RH]]),
                        _sub(u2T[:], f0 + i, [[ppu, BL], [I, RH]]),
                        _sub(srt[:], r0, [[ppg, BL], [1, RH]]), op=MUL)

            with tc.tile_pool(name="pH", bufs=1) as pH:
                h1 = [pH.tile([128, BL * JC], BF16, tag=f"h1_{kc}",
                              name=f"h1_{kc}") for kc in range(2)]

                # ===== Phase A: conv1 =====
                with tc.tile_pool(name="pAx", bufs=3) as pAx, \
                     tc.tile_pool(name="ps1", bufs=6, space="PSUM") as ps1:
                    ci = 0
                    for ch in range(NXC):
                        Ax = pAx.tile([81, XC * 784], BF16, tag="Ax")
                        src = _sub(xin, ch * XC * 784,
                                   [[28, 9], [1, 9], [1, XC * 784]])
                        (POOL if ch % 2 == 0 else SYNC).dma_start(Ax[:], src)
                        if ch == 3:
                            # small consts ride on the Pool queue
                            POOL.dma_start(b2_sb[:], b2)
                            POOL.dma_start(selA_sb[:], selA)
                            POOL.dma_start(selR_sb[:], selR)
                            POOL.dma_start(eyeb_sb[:], eyeb)
                        for mc in range(2):
                            lhsT = w1t_sb[:, mc * 128:(mc + 1) * 128]
                            for i in range(XC):
                                ps = ps1.tile([128, JC], F32, tag="c1ps")
                                rhs = _sub(Ax[:], i * 784,
                                           [[_pp(Ax[:]), 81], [28, 20], [1, 20]])
                                nc.tensor.matmul(ps[:], lhsT, rhs,
                                                 start=True, stop=True)
                                dst = h1[mc][:, (ch * XC + i) * JC:
                                             (ch * XC + i + 1) * JC]
                                relu_copy(PAT[ci % len(PAT)], dst, ps[:],
                                          b1_sb[:, mc:mc + 1])
                                ci += 1

                # ===== Phase B: conv2 (+ pipelined half-squash) =====
                with tc.tile_pool(name="pW2", bufs=2) as pW2, \
                     tc.tile_pool(name="pV", bufs=1) as pV, \
                     tc.tile_pool(name="ps2", bufs=1, space="PSUM") as ps2:
                    for mc in range(2):
                        pss = [ps2.tile([128, nb * S2], F32, tag=f"c2ps{i}",
                                        name=f"c2ps{i}_{mc}")
                               for i, (_, nb) in enumerate(BCH)]
                        wcbs = {}
                        wcbs[0] = pW2.tile([128, QW * 128], BF16, tag="wch",
                                           name=f"wch{mc}_0")
                        nc.sync.dma_start(wcbs[0][:], w2m[mc * NWCH])
                        for q in range(162):
                            ch, j = q // QW, q % QW
                            if j == 0 and ch + 1 < NWCH:
                                wcbs[ch + 1] = pW2.tile(
                                    [128, QW * 128], BF16, tag="wch",
                                    name=f"wch{mc}_{ch + 1}")
                                nc.sync.dma_start(wcbs[ch + 1][:],
                                                  w2m[mc * NWCH + ch + 1])
                            kc, khw = q // 81, q % 81
                            kh2, kw2 = khw // 9, khw % 9
                            lhsT = wcbs[ch][:, j * 128:(j + 1) * 128]
                            for ic, (b0, nb) in enumerate(BCH):
                                rhs = _sub(h1[kc][:],
                                           b0 * JC + 20 * kh2 + kw2,
                                           [[_pp(h1[kc][:]), 128],
                                            [JC, nb], [40, 6], [2, 6]])
                                nc.tensor.matmul(
                                    pss[ic][:], lhsT, rhs,
                                    start=(q == 0), stop=(q == 161))
                        v = pV.tile([128, N2], BF16, tag="v")
                        for ic, (b0, nb) in enumerate(BCH):
                            DVE.tensor_scalar(v[:, b0 * S2:(b0 + nb) * S2],
                                              pss[ic][:], b2_sb[:, mc:mc + 1],
                                              None, op0=ADD)
                        # transposed scatter-write to DRAM (u2T layout), then
                        # contiguous readback; both on the idle Act queue
                        vdst = _sub(vd2.ap(), mc * FH,
                                    [[S2, 128], [F, BL], [1, S2]])
                        SCA.dma_start(vdst, v[:])
                        vsrc = _sub(vd2.ap(), mc * FH, [[F, BL], [1, FH]])
                        SCA.dma_start(u2T[:, mc * FH:(mc + 1) * FH], vsrc)
                        if mc == 0:
                            # W for routing: load during conv2 mc=1 on Pool
                            POOL.dma_start(wsb[:], wlbr)
                        # squash this half on DVE/Act/Pool; for mc=0 it runs
                        # underneath mc=1's matmuls (those engines are idle)
                        squash_half(mc)

        if getattr(nc, "_nit", NIT) == 0:
            POOL.dma_start(out, u2T[:, 0:CO])   # junk, timing probe only
            return

        # ===== u2R = transpose(u2T) =====
        with tc.tile_pool(name="ptr", bufs=2, space="PSUM") as ptr:
            TRE = [DVE, SCA]
            for tb in range(KT // 4):
                pst = ptr.tile([128, 4 * BL], BF16, tag="tr")
                for s in range(4):
                    t = tb * 4 + s
                    nc.tensor.transpose(pst[:, s * BL:(s + 1) * BL],
                                        u2T[:, t * 128:(t + 1) * 128],
                                        eyeb_sb[:])
                eng = TRE[tb % 2]
                if eng is SCA:
                    SCA.activation(u2R[:, tb * 4 * BL:(tb + 1) * 4 * BL],
                                   pst[:], ACT.Copy)
                else:
                    eng.tensor_copy(u2R[:, tb * 4 * BL:(tb + 1) * 4 * BL],
                                    pst[:])

        # ===== routing =====
        with tc.tile_pool(name="pB", bufs=1) as pB, \
             tc.tile_pool(name="psq2", bufs=1) as pq, \
             tc.tile_pool(name="psT", bufs=2, space="PSUM") as psT, \
             tc.tile_pool(name="psM", bufs=1, space="PSUM") as psM, \
             tc.tile_pool(name="psB2", bufs=1, space="PSUM") as psB2, \
             tc.tile_pool(name="psN", bufs=1, space="PSUM") as psN, \
             tc.tile_pool(name="psS", bufs=1, space="PSUM") as psS:

            pm = pB.tile([128, KT * CO], BF16, tag="pm")
            wp = pB.tile([128, KT * CO], BF16, tag="wp")
            mAll = pB.tile([16, KT * C], F32, tag="mAll")
            msumT = pB.tile([16, KT * C], F32, tag="msumT")
            cEn = pB.tile([128, KT * C], BF16, tag="cEn")
            rrcB = pB.tile([BL, C], F32, tag="rrcB")
            rrc2 = pB.tile([BL, C], F32, tag="rrc2")
            tpsb = pB.tile([128, 3 * CO], BF16, tag="tpsb")
            v2T = pB.tile([BL, CO], F32, tag="v2T")
            v2Tb = pB.tile([BL, CO], BF16, tag="v2Tb")

            bFp = psB2.tile([128, KT * C], F32, tag="bFp", name="bFp")
            lam = 1.0 / R
            ppw = _pp(wp[:])
            pps = _pp(wsb[:])
            ppm = _pp(pm[:])
            ppe = _pp(cEn[:])
            for it in range(getattr(nc, "_nit", NIT)):
                if it > 0:
                    # unnormalized routing weights, f-major:
                    # cEn[p,(tt,c)] = exp(b[c, 16tt+p//8]); the softmax
                    # 1/sum normalizer is folded into the v-squash scale.
                    # b accumulates in PSUM (bFp); exp reads PSUM directly.
                    SCA.activation(cEn[:], bFp[:], ACT.Exp)
                    # rsum[c] = sum_{p,t} cEn/8 via 72 accumulating 10-col
                    # matmuls (ones lhsT loads once), then 1/rsum broadcast
                    # to 64 partitions with a rank-1 matmul
                    rsp = psN.tile([BL, C], F32, tag="nrm", name="rs_ps")
                    for t in range(KT):
                        nc.tensor.matmul(rsp[0:1, :], on128[:],
                                         cEn[:, t * C:(t + 1) * C],
                                         start=(t == 0), stop=(t == KT - 1))
                    rs1 = pq.tile([1, C], F32, tag="rs1")
                    DVE.reciprocal(rs1[:], rsp[0:1, :])
                    rbp = psN.tile([BL, C], F32, tag="nrm", name="rb_ps")
                    nc.tensor.matmul(rbp[:], on64[:], rs1[:],
                                     start=True, stop=True)
                    DVE.tensor_copy(rrcB[:], rbp[:])
                    DVE.tensor_mul(rrc2[:], rrcB[:], rrcB[:])
                    # wp = wsb * cEn, one pass per o (c contiguous)
                    for o in range(O):
                        eng = DVE if o % 4 < 3 else POOL
                        eng.tensor_tensor(
                            _sub(wp[:], o * C, [[ppw, 128], [CO, KT], [1, C]]),
                            _sub(wsb[:], o * C, [[pps, 128], [CO, KT], [1, C]]),
                            cEn[:].rearrange("p (t c) -> p t c", c=C), op=MUL)

                # s_j^T [b, (o,c)] over 72 accumulating K-tiles
                wcur = wsb if it == 0 else wp
                ssum = psS.tile([BL, CO], F32, tag="ssum")
                for t in range(KT):
                    nc.tensor.matmul(ssum[:], u2R[:, t * BL:(t + 1) * BL],
                                     wcur[:, t * CO:(t + 1) * CO],
                                     start=(t == 0), stop=(t == KT - 1))

                # v2 = squash(s) over o-groups; iter0 folds the 1/R scale,
                # iter>0 folds the softmax normalizer 1/rsum[c]
                svr = pq.tile([BL, CO], F32, tag="svr")
                SCA.activation(svr[:], ssum[:], ACT.Square)
                sqv = pq.tile([BL, C], F32, tag="sqv")
                nc.vector.tensor_reduce(
                    sqv[:], _sub(svr[:], 0, [[_pp(svr[:]), BL], [1, C], [C, O]]),
                    axis=AXX, op=ADD)
                if it == 0:
                    nc.vector.tensor_scalar(sqv[:], sqv[:], lam * lam, None,
                                            op0=MUL)
                else:
                    DVE.tensor_mul(sqv[:], sqv[:], rrc2[:])
                srtv = pq.tile([BL, C], F32, tag="srtv")
                SCA.sqrt(srtv[:], sqv[:])
                dv1 = pq.tile([BL, C], F32, tag="dv1")
                POOL.tensor_scalar(dv1[:], sqv[:], 1.0, None, op0=ADD)
                rcv = pq.tile([BL, C], F32, tag="rcv")
                DVE.reciprocal(rcv[:], dv1[:])
                gv = pq.tile([BL, C], F32, tag="gv")
                DVE.tensor_mul(gv[:], srtv[:], rcv[:])
                if it == 0:
                    nc.vector.tensor_scalar(gv[:], gv[:], lam, None, op0=MUL)
                else:
                    DVE.tensor_mul(gv[:], gv[:], rrcB[:])
                for o in range(O):
                    DVE.tensor_tensor(v2T[:, o * C:(o + 1) * C],
                                      ssum[:, o * C:(o + 1) * C],
                                      gv[:], op=MUL)

                if it == getattr(nc, "_nit", NIT) - 1:
                    # v2T is (o,c); host un-permutes to (c,o)
                    nc.sync.dma_start(out, v2T[:])
                    break

                nc.vector.tensor_copy(v2Tb[:], v2T[:])
                # agreement: tpsT[f,(o,c)] = sum_b u2T[b,f] v2Tb[b,:]
                for rd in range(KT // 3):
                    tps = psT.tile([128, 3 * CO], F32, tag="tps")
                    for s in range(3):
                        t = rd * 3 + s
                        nc.tensor.matmul(tps[:, s * CO:(s + 1) * CO],
                                         u2T[:, t * 128:(t + 1) * 128],
                                         v2Tb[:], start=True, stop=True)
                    if rd % 3 < 2:
                        DVE.tensor_tensor(
                            pm[:, rd * 3 * CO:(rd + 1) * 3 * CO],
                            wsb[:, rd * 3 * CO:(rd + 1) * 3 * CO],
                            tps[:], op=MUL)
                    else:
                        SCA.activation(tpsb[:], tps[:], ACT.Copy)
                        POOL.tensor_tensor(
                            pm[:, rd * 3 * CO:(rd + 1) * 3 * CO],
                            wsb[:, rd * 3 * CO:(rd + 1) * 3 * CO],
                            tpsb[:], op=MUL)
                # m16[r', (t36,c)] = sum_{i,o} pm (i via selA, o via accum)
                for hf in range(2):
                    m16 = psM.tile([16, 36 * C], F32, tag=f"m16{hf}",
                                   name=f"m16{hf}")
                    for o in range(O):
                        rhs = _sub(pm[:], hf * 36 * CO + o * C,
                                   [[ppm, 128], [CO, 36], [1, C]])
                        nc.tensor.matmul(m16[:], selA_sb[:], rhs,
                                         start=(o == 0), stop=(o == O - 1))
                    if hf == 0:
                        DVE.tensor_copy(mAll[:, 0:36 * C], m16[:])
                    else:
                        SCA.activation(mAll[:, 36 * C:72 * C], m16[:],
                                       ACT.Copy)
                # AllReduce in the mAll layout [r', (tt,c)]
                nc.sync.dma_start(cc_in.ap(), mAll[:])
                if getattr(nc, "_for_sim", False):
                    nc.sync.dma_start(cc_out.ap(), cc_in.ap())
                else:
                    nc.gpsimd.collective_compute(
                        "AllReduce", ADD,
                        replica_groups=[list(range(NCORES))],
                        ins=[cc_in.ap()], outs=[cc_out.ap()])
                nc.sync.dma_start(msumT[:], cc_out.ap())
                # bFp[p,(tt,c)] += b_ij[c, 16tt+p//8] via replication matmul
                # (selR carries the 1/B scale; accumulates across iters)
                nits = getattr(nc, "_nit", NIT)
                nc.tensor.matmul(bFp[:, 0:512], selR_sb[:], msumT[:, 0:512],
                                 start=(it == 0), stop=(it == nits - 2),
                                 skip_group_check=True)
                nc.tensor.matmul(bFp[:, 512:KT * C], selR_sb[:],
                                 msumT[:, 512:KT * C], start=(it == 0),
                                 stop=(it == nits - 2),
                                 skip_group_check=True)


# ------------------------- host side ---------------------------------------
_CACHE = {}


def pack_inputs(x, conv1_w, conv1_b, conv2_w, conv2_b, W):
    """Build the shared weight map + per-core xin slices."""
    bf = ml_dtypes.bfloat16
    xf = np.ascontiguousarray(np.asarray(x, np.float32).reshape(B, 784))
    w1 = np.ascontiguousarray(
        np.asarray(conv1_w, np.float32).reshape(256, 81).T).astype(bf)
    b1v = np.asarray(conv1_b, np.float32).reshape(2, 128).T.copy()
    a = np.asarray(conv2_w, np.float32).reshape(2, 128, 2, 128, 81)
    a = a.transpose(0, 2, 4, 3, 1).reshape(2, 162, 128, 128)
    w2 = np.ascontiguousarray(
        a.reshape(2, NWCH, QW, 128, 128).transpose(0, 1, 3, 2, 4)
    ).reshape(2 * NWCH, 128, QW * 128).astype(bf)
    b2v = np.asarray(conv2_b, np.float32).reshape(2, 128).T.copy()
    Wf = np.asarray(W, np.float32)
    wl = np.ascontiguousarray(Wf.transpose(0, 3, 2, 1)).reshape(F, CO)
    wlr = np.ascontiguousarray(
        wl.reshape(KT, 128, CO).transpose(1, 0, 2)).reshape(128, KT * CO).astype(bf)
    sA = np.zeros((128, 16), np.float32)
    sA[np.arange(128), np.arange(128) // 8] = 1.0
    sR = np.ascontiguousarray(sA.T / B).astype(np.float32)
    shared = {
        "w1t": w1, "b1": b1v, "w2m": w2, "b2": b2v, "wlbr": wlr,
        "selA": sA.astype(bf), "selR": sR,
        "eyeb": np.eye(64).astype(bf),
    }
    xins = []
    for c in range(NCORES):
        xs = np.zeros(BL * 784 + 256, bf)
        xs[:BL * 784] = xf[c * BL:(c + 1) * BL].reshape(-1).astype(bf)
        xins.append(xs)
    return shared, xins


def kernel(x, conv1_w, conv1_b, conv2_w, conv2_b, W):
    if "nc" not in _CACHE:
        _CACHE["nc"] = build_nc()
    nc = _CACHE["nc"]
    shared, xins = pack_inputs(x, conv1_w, conv1_b, conv2_w, conv2_b, W)
    in_maps = [{"xin": xins[c], **shared} for c in range(NCORES)]
    res = run_bass_kernel_spmd(nc, in_maps, list(range(NCORES)), trace=False)
    outs = [res.results[c]["out"] for c in range(NCORES)]
    full = np.concatenate(outs, axis=0).reshape(B, O, C)
    return np.ascontiguousarray(full.transpose(0, 2, 1)).astype(np.float32)
